# revision 10
# baseline (speedup 1.0000x reference)
"""Trainium2 Bass kernel for nn_DiffFDN: H(z) = C^T (diag(z^m) - A*Gamma)^-1 B
evaluated on F=192001 frequencies, plus h = normalized irfft of sum(H*C).

Strategy (per sharding hint): shard the frequency axis across 8 NeuronCores
(pure data parallel, 24064 freqs/core laid out as [128 partitions x 188 cols]).
Each core evaluates the per-frequency diagonal d_i = z^{m_i} with ACT Sin
(host supplies exactly range-reduced phase arguments via integer arithmetic)
and solves the 8x8 complex system per frequency with a fully unrolled,
unpivoted augmented Gaussian elimination on the vector engines (safe: the
leading principal minors of D - A*Gamma are bounded away from 0 because
||A*Gamma||_2 <= max(gamma) < 1 and |d_i| = 1).

Host only does O(N)/O(F) integer bookkeeping, the tiny 8x8 expm, and the
final irfft + normalization of the gathered F-vector.
"""

import numpy as np

# ---- module constants of the reference nn.Module (not inputs) ----
_DELAYS = np.array([809., 877., 937., 1049., 1151., 1249., 1373., 1499.],
                   dtype=np.float32)
N = 8
F = 192001
NFFT = 384000
HALF = 192000           # NFFT // 2
GAIN_PER_SAMPLE = 0.9999
NCORES = 8
P = 128                 # SBUF partitions
T = 188                 # free-dim columns per plane
FC = P * T              # 24064 freqs per core
FPAD = FC * NCORES      # 192512

_PI_SAFE = float(np.nextafter(np.float32(np.pi), np.float32(0.0)))


def _host_params(B, C, W, m):
    """Tiny N=8 parameter prep (mirrors the reference's fp32 arithmetic)."""
    M_AVR = _DELAYS.mean(dtype=np.float32)
    M_STD = _DELAYS.std(ddof=1, dtype=np.float32)
    md = (m.astype(np.float32) * np.float32(M_STD) + np.float32(M_AVR)).astype(np.float32)
    gamma = np.power(np.float32(GAIN_PER_SAMPLE), md).astype(np.float32)
    # A = expm(S - S^T) via eigendecomposition of the Hermitian i*(S - S^T)
    S = np.triu(W.astype(np.float64), 1)
    K = S - S.T
    lam, V = np.linalg.eigh(1j * K)
    A = (V @ np.diag(np.exp(-1j * lam)) @ V.conj().T).real.astype(np.float32)
    AG = (A * gamma[None, :]).astype(np.float32)
    return md, AG


def _theta_planes(x, md):
    """[16, FPAD] float32 phase planes, exactly reduced to [-pi, pi].

    Plane 2i   -> argument of cos(md_i * w)  (as sin(arg + pi/2))
    Plane 2i+1 -> argument of sin(md_i * w)
    """
    md64 = md.astype(np.float64)
    m_int = np.round(md64).astype(np.int64)
    res = md64 - m_int

    f_idx = np.arange(FPAD, dtype=np.int64)
    f_idx[F:] = HALF  # padding tail: any valid frequency

    # Verify x matches the canonical grid exp(i*pi*f/192000); if not, fall
    # back to computing phases from angle(x) directly (still exact reduction).
    grid_ok = False
    xc = np.asarray(x)
    if xc.shape == (F,) and np.iscomplexobj(xc):
        wg = np.pi * np.arange(F, dtype=np.float64) / HALF
        dev = np.max(np.abs(xc.astype(np.complex128) - np.exp(1j * wg)))
        grid_ok = bool(dev < 1e-4)

    theta = np.empty((16, FPAD), dtype=np.float64)
    if grid_ok:
        t_sin = (f_idx[None, :] * m_int[:, None] + HALF) % NFFT - HALF
        t_cos = (f_idx[None, :] * m_int[:, None] + 96000 + HALF) % NFFT - HALF
        theta[0::2] = t_cos * (np.pi / HALF)
        theta[1::2] = t_sin * (np.pi / HALF)
        if np.any(res != 0.0):
            w = (np.pi / HALF) * f_idx.astype(np.float64)
            theta[0::2] += w[None, :] * res[:, None]
            theta[1::2] += w[None, :] * res[:, None]
            theta = (theta + np.pi) % (2 * np.pi) - np.pi
    else:
        w = np.angle(xc.astype(np.complex128))
        w = np.concatenate([w, np.full(FPAD - F, w[-1])])
        base = md64[:, None] * w[None, :]
        theta[0::2] = (base + np.pi / 2 + np.pi) % (2 * np.pi) - np.pi
        theta[1::2] = (base + np.pi) % (2 * np.pi) - np.pi

    return np.clip(theta, -_PI_SAFE, _PI_SAFE).astype(np.float32)


def _build_nc(AG, Bv, Cv, pool_rows=(3, 6), cmax=4, wbufs=3, abufs=4, w12bufs=3, split_paths=False,
               pool_wb=(), tchain_dve=False):
    """Build the single-core Bass/Tile program (SPMD across 8 cores)."""
    import concourse.bacc as bacc
    import concourse.bass as bass
    import concourse.mybir as mybir
    import concourse.tile as tile

    f32 = mybir.dt.float32
    AF = mybir.ActivationFunctionType
    op = mybir.AluOpType
    T2 = 2 * T

    AGf = [[float(AG[i, j]) for j in range(N)] for i in range(N)]
    Bf = [float(Bv[i]) for i in range(N)]
    Cf = [float(Cv[i]) for i in range(N)]
    # step-0 fold constants: l_i = AG[i,0]*q with q = -1/(d0 - AG00)
    Pq = [[AGf[i][0] * AGf[0][j] for j in range(N)] for i in range(N)]
    PB = [AGf[i][0] * Bf[0] for i in range(N)]

    nc = bacc.Bacc(None)
    th_d = nc.dram_tensor("theta", [P, 16 * T], f32, kind="ExternalInput")
    H_d = nc.dram_tensor("H", [P, 16 * T], f32, kind="ExternalOutput")
    S_d = nc.dram_tensor("S", [P, T2], f32, kind="ExternalOutput")

    def eng(i):
        # split the independent per-row work across DVE and GPSIMD
        return nc.gpsimd if i in pool_rows else nc.vector

    with tile.TileContext(nc) as tc:
        with tc.tile_pool(name="main", bufs=1) as pool:
            def stt2(e, out, in0, scalar, in1, op0, op1):
                # (in0 op0 scalar) op1 in1 without the STT ISA struct
                # (walrus rejects STT when Tile needs >1 sync wait on it)
                w = out.shape[-1]
                tmp = pool.tile([P, w], f32, name="sttmp", tag=f"sttmp{w}", bufs=abufs)
                e.tensor_scalar(tmp[:], in0, scalar, None, op0)
                e.tensor_tensor(out, tmp[:], in1, op1)

            th = pool.tile([P, 16 * T], f32, name="th", tag="io")
            nc.sync.dma_start(th[:], th_d[:])

            # d planes: block 2i = cos_i (re), 2i+1 = sin_i (im)
            d = pool.tile([P, 16 * T], f32, name="d", tag="dy")
            for q in range(16):
                nc.scalar.activation(d[:, q * T:(q + 1) * T],
                                     th[:, q * T:(q + 1) * T], AF.Sin)

            def dre(i):
                return d[:, (2 * i) * T:(2 * i + 1) * T]

            def dim(i):
                return d[:, (2 * i + 1) * T:(2 * i + 2) * T]

            # augmented rows i=1..7, cols j=1..8 (8 = RHS), packed re|im
            Mrow = [None] + [pool.tile([P, 8 * T2], f32, name=f"Mrow{i}", tag=f"M{i}")
                             for i in range(1, N)]

            def MR(i, j):
                return Mrow[i][:, (j - 1) * T2:(j - 1) * T2 + T]

            def MI(i, j):
                return Mrow[i][:, (j - 1) * T2 + T:j * T2]

            def MP(i, j, c=1):
                return Mrow[i][:, (j - 1) * T2:(j - 1 + c) * T2]

            # ---- step 0 (pivot row 0 is constant: M0j = -AG0j, b0 = B0) ----
            pr = pool.tile([P, T], f32, name="pr", tag="pr")
            nc.vector.tensor_scalar(pr[:], dre(0), AGf[0][0], None, op.subtract)
            pi = dim(0)
            den = pool.tile([P, T], f32, name="den", tag="den", bufs=2)
            m1 = pool.tile([P, T], f32, name="m1", tag="m1", bufs=2)
            nc.vector.tensor_tensor(den[:], pr[:], pr[:], op.mult)
            nc.vector.tensor_tensor(m1[:], pi, pi, op.mult)
            nc.vector.tensor_tensor(den[:], den[:], m1[:], op.add)
            rec = pool.tile([P, T], f32, name="rec", tag="rec", bufs=2)
            nc.vector.reciprocal(rec[:], den[:])
            qr = pool.tile([P, T], f32, name="qr", tag="qr")
            qi = pool.tile([P, T], f32, name="qi", tag="qi")
            stt2(nc.vector, qr[:], pr[:], -1.0, rec[:], op.mult, op.mult)
            nc.vector.tensor_tensor(qi[:], pi, rec[:], op.mult)

            # diag shift: dre_i -= AG_ii (in place, ACT)
            for i in range(1, N):
                nc.scalar.activation(dre(i), dre(i), AF.Copy, bias=-AGf[i][i])

            # M_ij^(1) = -AG_ij + Pq_ij*q (+ d_i on diag); RHS: B_i - PB_i*q
            for i in range(1, N):
                e = eng(i)
                for j in range(1, N):
                    if j == i:
                        stt2(e, MR(i, i), qr[:], Pq[i][i], dre(i), op.mult, op.add)
                        stt2(e, MI(i, i), qi[:], Pq[i][i], dim(i), op.mult, op.add)
                    else:
                        h = (i * 8 + j) % 5
                        if h < 2:
                            nc.scalar.activation(MR(i, j), qr[:], AF.Copy,
                                                 scale=Pq[i][j], bias=-AGf[i][j])
                            nc.scalar.activation(MI(i, j), qi[:], AF.Copy, scale=Pq[i][j])
                        else:
                            e2 = nc.vector if h < 4 else nc.gpsimd
                            e2.tensor_scalar(MR(i, j), qr[:], Pq[i][j], -AGf[i][j],
                                             op.mult, op.add)
                            e2.tensor_scalar(MI(i, j), qi[:], Pq[i][j], None, op.mult)
                nc.vector.tensor_scalar(MR(i, 8), qr[:], -PB[i], Bf[i], op.mult, op.add)
                nc.scalar.activation(MI(i, 8), qi[:], AF.Copy, scale=-PB[i])

            # persistent pivot inverses (reused in back-substitution)
            inv = [None] * N

            def make_inv(k):
                deni = pool.tile([P, T], f32, name="deni", tag="den", bufs=2)
                mm = pool.tile([P, T], f32, name="mm", tag="m1", bufs=2)
                nc.vector.tensor_tensor(deni[:], MR(k, k), MR(k, k), op.mult)
                nc.vector.tensor_tensor(mm[:], MI(k, k), MI(k, k), op.mult)
                nc.vector.tensor_tensor(deni[:], deni[:], mm[:], op.add)
                reci = pool.tile([P, T], f32, name="reci", tag="rec", bufs=2)
                nc.vector.reciprocal(reci[:], deni[:])
                inv[k] = pool.tile([P, T2], f32, name=f"inv{k}", tag=f"inv{k}")
                nc.vector.tensor_tensor(inv[k][:, :T], MR(k, k), reci[:], op.mult)
                stt2(nc.vector, inv[k][:, T:], MI(k, k), -1.0,
                     reci[:], op.mult, op.mult)

            # ---- elimination steps k=1..6 ----
            CMAX = cmax  # wide-span cap (SBUF scratch sizing)
            for k in range(1, 7):
                make_inv(k)
                for i in range(k + 1, N):
                    e = eng(i)
                    et = nc.vector if tchain_dve else e
                    # t_i = M_ik * inv_k  (complex, packed into tpk)
                    tpk = pool.tile([P, T2], f32, name="tpk", tag="tpk", bufs=abufs)
                    w1 = pool.tile([P, T2], f32, name="w1", tag="w1", bufs=w12bufs)
                    w2 = pool.tile([P, T2], f32, name="w2", tag="w2", bufs=w12bufs)
                    iv = inv[k][:]
                    ivs = iv.rearrange("p (two t) -> p two t", two=2, t=T)[:, ::-1, :]
                    et.tensor_tensor(w1[:], MP(i, k), iv, op.mult)
                    et.tensor_tensor(w2[:].rearrange("p (two t) -> p two t", two=2, t=T),
                                    MP(i, k).rearrange("p (two t) -> p two t", two=2, t=T),
                                    ivs, op.mult)
                    et.tensor_tensor(tpk[:, :T], w1[:, :T], w1[:, T:], op.subtract)
                    et.tensor_tensor(tpk[:, T:], w2[:, :T], w2[:, T:], op.add)
                    tb3 = tpk[:].unsqueeze(1)
                    tb4 = tpk[:].rearrange("p (two t) -> p two t", two=2).unsqueeze(1)
                    # wide span updates, chunked to <= CMAX column blocks
                    j = k + 1
                    while j <= 8:
                        c = min(CMAX, 8 - j + 1)
                        si4 = MP(i, j, c).rearrange("p (c two t) -> p c two t",
                                                    two=2, t=T)
                        sk3 = MP(k, j, c).rearrange("p (c x) -> p c x", c=c)
                        sk4s = MP(k, j, c).rearrange("p (c two t) -> p c two t",
                                                    two=2, t=T)[:, :, ::-1, :]
                        wA = pool.tile([P, c * T2], f32, name="wA", tag="wA", bufs=wbufs)
                        wB = pool.tile([P, c * T2], f32, name="wB", tag="wB", bufs=wbufs)
                        wA3 = wA[:].rearrange("p (c x) -> p c x", c=c)
                        wA4 = wA[:].rearrange("p (c two t) -> p c two t", two=2, t=T)
                        wB4 = wB[:].rearrange("p (c two t) -> p c two t", two=2, t=T)
                        eB = nc.gpsimd if (split_paths and e is nc.vector
                                           and i % 2 == 0) or i in pool_wb else e
                        e.tensor_tensor(wA3, sk3, tb3.broadcast_to([P, c, T2]), op.mult)
                        eB.tensor_tensor(wB4, sk4s, tb4.broadcast_to([P, c, 2, T]), op.mult)
                        e.tensor_tensor(wA4[:, :, 0, :], wA4[:, :, 0, :],
                                        wA4[:, :, 1, :], op.subtract)
                        eB.tensor_tensor(wB4[:, :, 0, :], wB4[:, :, 0, :],
                                        wB4[:, :, 1, :], op.add)
                        e.tensor_tensor(si4[:, :, 0, :], si4[:, :, 0, :],
                                        wA4[:, :, 0, :], op.subtract)
                        eB.tensor_tensor(si4[:, :, 1, :], si4[:, :, 1, :],
                                        wB4[:, :, 0, :], op.subtract)
                        j += c
            make_inv(7)

            # ---- back-substitution ----
            y = pool.tile([P, 8 * T2], f32, name="y", tag="dy")  # reuses d's slot

            def yP(i):
                return y[:, i * T2:(i + 1) * T2]

            def yR(i):
                return y[:, i * T2:i * T2 + T]

            def yI(i):
                return y[:, i * T2 + T:(i + 1) * T2]

            def divide(i):
                # y_i = M_i8 * inv_i  (packed: 4 ops)
                b1 = pool.tile([P, T2], f32, name="b1", tag="w1", bufs=w12bufs)
                b2 = pool.tile([P, T2], f32, name="b2", tag="w2", bufs=w12bufs)
                iv = inv[i][:]
                ivs = iv.rearrange("p (two t) -> p two t", two=2, t=T)[:, ::-1, :]
                nc.vector.tensor_tensor(b1[:], MP(i, 8), iv, op.mult)
                nc.vector.tensor_tensor(b2[:].rearrange("p (two t) -> p two t", two=2, t=T),
                                        MP(i, 8).rearrange("p (two t) -> p two t", two=2, t=T),
                                        ivs, op.mult)
                nc.vector.tensor_tensor(yR(i), b1[:, :T], b1[:, T:], op.subtract)
                nc.vector.tensor_tensor(yI(i), b2[:, :T], b2[:, T:], op.add)

            divide(7)
            for jj in range(7, 1, -1):
                for i in range(jj - 1, 0, -1):
                    e = eng(i)
                    # M_i8 -= M_i,jj * y_jj
                    pA = pool.tile([P, T2], f32, name="pA", tag="pA", bufs=abufs)
                    pB = pool.tile([P, T2], f32, name="pB", tag="pB", bufs=abufs)
                    e.tensor_tensor(pA[:], MP(i, jj), yP(jj), op.mult)
                    Msw = MP(i, jj).rearrange("p (two t) -> p two t", two=2)[:, ::-1, :]
                    e.tensor_tensor(pB[:].rearrange("p (two t) -> p two t", two=2),
                                    Msw, yP(jj).rearrange("p (two t) -> p two t", two=2),
                                    op.mult)
                    e.tensor_tensor(pA[:, :T], pA[:, :T], pA[:, T:], op.subtract)
                    e.tensor_tensor(pA[:, T:], pB[:, :T], pB[:, T:], op.add)
                    e.tensor_tensor(MP(i, 8), MP(i, 8), pA[:], op.subtract)
                divide(jj - 1)

            # y_0 = -q * (B_0 + sum_j AG_0j * y_j)
            acc0 = pool.tile([P, T2], f32, name="acc0", tag="acc0")
            nc.scalar.activation(acc0[:, :T], yR(1), AF.Copy, scale=AGf[0][1], bias=Bf[0])
            nc.scalar.activation(acc0[:, T:], yI(1), AF.Copy, scale=AGf[0][1])
            for j in range(2, N):
                stt2(nc.vector, acc0[:], yP(j), AGf[0][j], acc0[:],
                     op.mult, op.add)
            c1 = pool.tile([P, T], f32, name="c1", tag="w1", bufs=w12bufs)
            c2 = pool.tile([P, T], f32, name="c2", tag="w2", bufs=w12bufs)
            nc.vector.tensor_tensor(c1[:], qi[:], acc0[:, T:], op.mult)
            nc.vector.tensor_tensor(c2[:], qr[:], acc0[:, :T], op.mult)
            nc.vector.tensor_tensor(yR(0), c1[:], c2[:], op.subtract)
            nc.vector.tensor_tensor(c1[:], qr[:], acc0[:, T:], op.mult)
            nc.vector.tensor_tensor(c2[:], qi[:], acc0[:, :T], op.mult)
            nc.vector.tensor_tensor(c1[:], c1[:], c2[:], op.add)
            nc.vector.tensor_scalar(yI(0), c1[:], -1.0, None, op.mult)

            # ---- outputs: H_i = C_i * y_i ; S = sum_i H_i ----
            Hout = pool.tile([P, 16 * T], f32, name="Hout", tag="io")  # reuses theta's slot

            def HP(i):
                return Hout[:, i * T2:(i + 1) * T2]

            for i in range(N):
                e3 = nc.gpsimd if i in (2, 5) else (
                    nc.vector if i % 2 == 0 else nc.vector)
                e3.tensor_scalar(HP(i), yP(i), Cf[i], None, op.mult)
            s01 = pool.tile([P, T2], f32, name="s01", tag="s01")
            s23 = pool.tile([P, T2], f32, name="s23", tag="s23")
            s45 = pool.tile([P, T2], f32, name="s45", tag="s45")
            s67 = pool.tile([P, T2], f32, name="s67", tag="s67")
            nc.vector.tensor_tensor(s01[:], HP(0), HP(1), op.add)
            nc.gpsimd.tensor_tensor(s23[:], HP(2), HP(3), op.add)
            nc.vector.tensor_tensor(s45[:], HP(4), HP(5), op.add)
            nc.gpsimd.tensor_tensor(s67[:], HP(6), HP(7), op.add)
            nc.vector.tensor_tensor(s01[:], s01[:], s23[:], op.add)
            nc.vector.tensor_tensor(s45[:], s45[:], s67[:], op.add)
            Sout = pool.tile([P, T2], f32, name="Sout", tag="Sout")
            nc.vector.tensor_tensor(Sout[:], s01[:], s45[:], op.add)

            nc.sync.dma_start(H_d[:], Hout[:])
            nc.sync.dma_start(S_d[:], Sout[:])

    nc.compile()
    return nc


def kernel(x, B, C, W, m):
    from concourse.bass_utils import run_bass_kernel_spmd

    B = np.asarray(B)
    C = np.asarray(C)
    W = np.asarray(W)
    m = np.asarray(m)

    md, AG = _host_params(B, C, W, m)
    theta = _theta_planes(x, md)  # [16, FPAD] f32

    # per-core input: [128, 16*T] with plane-major blocks, f_local = p*T + t
    in_maps = []
    for c in range(NCORES):
        chunk = theta[:, c * FC:(c + 1) * FC].reshape(16, P, T)
        in_maps.append({"theta": np.ascontiguousarray(
            chunk.transpose(1, 0, 2).reshape(P, 16 * T))})

    nc = _build_nc(AG, B.reshape(-1), C.reshape(-1))
    res = run_bass_kernel_spmd(nc, in_maps, list(range(NCORES)))

    H = np.empty((FPAD, N), dtype=np.complex64)
    Hs = np.empty(FPAD, dtype=np.complex64)
    for c in range(NCORES):
        o = res.results[c]["H"].reshape(P, 16, T)
        s = res.results[c]["S"].reshape(P, 2, T)
        sl = slice(c * FC, (c + 1) * FC)
        for i in range(N):
            H[sl, i] = (o[:, 2 * i, :] + 1j * o[:, 2 * i + 1, :]).reshape(FC)
        Hs[sl] = (s[:, 0, :] + 1j * s[:, 1, :]).reshape(FC)
    H = H[:F]
    Hs = Hs[:F]

    h = np.fft.irfft(Hs.astype(np.complex128), n=NFFT)
    h = (h / np.max(np.abs(h))).astype(np.float32)
    return H.astype(np.complex64), h


# revision 12
# speedup vs baseline: 1.0015x; 1.0015x over previous
"""Trainium2 Bass kernel for nn_DiffFDN: H(z) = C^T (diag(z^m) - A*Gamma)^-1 B
evaluated on F=192001 frequencies, plus h = normalized irfft of sum(H*C).

Strategy (per sharding hint): shard the frequency axis across 8 NeuronCores
(pure data parallel, 24064 freqs/core laid out as [128 partitions x 188 cols]).
Each core evaluates the per-frequency diagonal d_i = z^{m_i} with ACT Sin
(host supplies exactly range-reduced phase arguments via integer arithmetic)
and solves the 8x8 complex system per frequency with a fully unrolled,
unpivoted augmented Gaussian elimination on the vector engines (safe: the
leading principal minors of D - A*Gamma are bounded away from 0 because
||A*Gamma||_2 <= max(gamma) < 1 and |d_i| = 1).

Host only does O(N)/O(F) integer bookkeeping, the tiny 8x8 expm, and the
final irfft + normalization of the gathered F-vector.
"""

import numpy as np

# ---- module constants of the reference nn.Module (not inputs) ----
_DELAYS = np.array([809., 877., 937., 1049., 1151., 1249., 1373., 1499.],
                   dtype=np.float32)
N = 8
F = 192001
NFFT = 384000
HALF = 192000           # NFFT // 2
GAIN_PER_SAMPLE = 0.9999
NCORES = 8
P = 128                 # SBUF partitions
T = 188                 # free-dim columns per plane
FC = P * T              # 24064 freqs per core
FPAD = FC * NCORES      # 192512

_PI_SAFE = float(np.nextafter(np.float32(np.pi), np.float32(0.0)))


def _host_params(B, C, W, m):
    """Tiny N=8 parameter prep (mirrors the reference's fp32 arithmetic)."""
    M_AVR = _DELAYS.mean(dtype=np.float32)
    M_STD = _DELAYS.std(ddof=1, dtype=np.float32)
    md = (m.astype(np.float32) * np.float32(M_STD) + np.float32(M_AVR)).astype(np.float32)
    gamma = np.power(np.float32(GAIN_PER_SAMPLE), md).astype(np.float32)
    # A = expm(S - S^T) via eigendecomposition of the Hermitian i*(S - S^T)
    S = np.triu(W.astype(np.float64), 1)
    K = S - S.T
    lam, V = np.linalg.eigh(1j * K)
    A = (V @ np.diag(np.exp(-1j * lam)) @ V.conj().T).real.astype(np.float32)
    AG = (A * gamma[None, :]).astype(np.float32)
    return md, AG


def _theta_planes(x, md):
    """[16, FPAD] float32 phase planes, exactly reduced to [-pi, pi].

    Plane 2i   -> argument of cos(md_i * w)  (as sin(arg + pi/2))
    Plane 2i+1 -> argument of sin(md_i * w)
    """
    md64 = md.astype(np.float64)
    m_int = np.round(md64).astype(np.int64)
    res = md64 - m_int

    f_idx = np.arange(FPAD, dtype=np.int64)
    f_idx[F:] = HALF  # padding tail: any valid frequency

    # Verify x matches the canonical grid exp(i*pi*f/192000); if not, fall
    # back to computing phases from angle(x) directly (still exact reduction).
    grid_ok = False
    xc = np.asarray(x)
    if xc.shape == (F,) and np.iscomplexobj(xc):
        wg = np.pi * np.arange(F, dtype=np.float64) / HALF
        dev = np.max(np.abs(xc.astype(np.complex128) - np.exp(1j * wg)))
        grid_ok = bool(dev < 1e-4)

    theta = np.empty((16, FPAD), dtype=np.float64)
    if grid_ok:
        t_sin = (f_idx[None, :] * m_int[:, None] + HALF) % NFFT - HALF
        t_cos = (f_idx[None, :] * m_int[:, None] + 96000 + HALF) % NFFT - HALF
        theta[0::2] = t_cos * (np.pi / HALF)
        theta[1::2] = t_sin * (np.pi / HALF)
        if np.any(res != 0.0):
            w = (np.pi / HALF) * f_idx.astype(np.float64)
            theta[0::2] += w[None, :] * res[:, None]
            theta[1::2] += w[None, :] * res[:, None]
            theta = (theta + np.pi) % (2 * np.pi) - np.pi
    else:
        w = np.angle(xc.astype(np.complex128))
        w = np.concatenate([w, np.full(FPAD - F, w[-1])])
        base = md64[:, None] * w[None, :]
        theta[0::2] = (base + np.pi / 2 + np.pi) % (2 * np.pi) - np.pi
        theta[1::2] = (base + np.pi) % (2 * np.pi) - np.pi

    return np.clip(theta, -_PI_SAFE, _PI_SAFE).astype(np.float32)


def _build_nc(AG, Bv, Cv, pool_rows=(3, 6), cmax=4, wbufs=3, abufs=4, w12bufs=3, split_paths=False,
               pool_wb=(), tchain_dve=False, karatsuba=False, kbufs=2):
    """Build the single-core Bass/Tile program (SPMD across 8 cores)."""
    import concourse.bacc as bacc
    import concourse.bass as bass
    import concourse.mybir as mybir
    import concourse.tile as tile

    f32 = mybir.dt.float32
    AF = mybir.ActivationFunctionType
    op = mybir.AluOpType
    T2 = 2 * T

    AGf = [[float(AG[i, j]) for j in range(N)] for i in range(N)]
    Bf = [float(Bv[i]) for i in range(N)]
    Cf = [float(Cv[i]) for i in range(N)]
    # step-0 fold constants: l_i = AG[i,0]*q with q = -1/(d0 - AG00)
    Pq = [[AGf[i][0] * AGf[0][j] for j in range(N)] for i in range(N)]
    PB = [AGf[i][0] * Bf[0] for i in range(N)]

    nc = bacc.Bacc(None)
    th_d = nc.dram_tensor("theta", [P, 16 * T], f32, kind="ExternalInput")
    H_d = nc.dram_tensor("H", [P, 16 * T], f32, kind="ExternalOutput")
    S_d = nc.dram_tensor("S", [P, T2], f32, kind="ExternalOutput")

    def eng(i):
        # split the independent per-row work across DVE and GPSIMD
        return nc.gpsimd if i in pool_rows else nc.vector

    with tile.TileContext(nc) as tc:
        with tc.tile_pool(name="main", bufs=1) as pool:
            def stt2(e, out, in0, scalar, in1, op0, op1):
                # (in0 op0 scalar) op1 in1 without the STT ISA struct
                # (walrus rejects STT when Tile needs >1 sync wait on it)
                w = out.shape[-1]
                tmp = pool.tile([P, w], f32, name="sttmp", tag=f"sttmp{w}", bufs=abufs)
                e.tensor_scalar(tmp[:], in0, scalar, None, op0)
                e.tensor_tensor(out, tmp[:], in1, op1)

            th = pool.tile([P, 16 * T], f32, name="th", tag="io")
            nc.sync.dma_start(th[:], th_d[:])

            # d planes: block 2i = cos_i (re), 2i+1 = sin_i (im)
            d = pool.tile([P, 16 * T], f32, name="d", tag="dy")
            for q in range(16):
                nc.scalar.activation(d[:, q * T:(q + 1) * T],
                                     th[:, q * T:(q + 1) * T], AF.Sin)

            def dre(i):
                return d[:, (2 * i) * T:(2 * i + 1) * T]

            def dim(i):
                return d[:, (2 * i + 1) * T:(2 * i + 2) * T]

            # augmented rows i=1..7, cols j=1..8 (8 = RHS), packed re|im
            Mrow = [None] + [pool.tile([P, 8 * T2], f32, name=f"Mrow{i}", tag=f"M{i}")
                             for i in range(1, N)]

            def MR(i, j):
                return Mrow[i][:, (j - 1) * T2:(j - 1) * T2 + T]

            def MI(i, j):
                return Mrow[i][:, (j - 1) * T2 + T:j * T2]

            def MP(i, j, c=1):
                return Mrow[i][:, (j - 1) * T2:(j - 1 + c) * T2]

            # ---- step 0 (pivot row 0 is constant: M0j = -AG0j, b0 = B0) ----
            pr = pool.tile([P, T], f32, name="pr", tag="pr")
            nc.vector.tensor_scalar(pr[:], dre(0), AGf[0][0], None, op.subtract)
            pi = dim(0)
            den = pool.tile([P, T], f32, name="den", tag="den", bufs=2)
            m1 = pool.tile([P, T], f32, name="m1", tag="m1", bufs=2)
            nc.vector.tensor_tensor(den[:], pr[:], pr[:], op.mult)
            nc.vector.tensor_tensor(m1[:], pi, pi, op.mult)
            nc.vector.tensor_tensor(den[:], den[:], m1[:], op.add)
            rec = pool.tile([P, T], f32, name="rec", tag="rec", bufs=2)
            nc.vector.reciprocal(rec[:], den[:])
            qr = pool.tile([P, T], f32, name="qr", tag="qr")
            qi = pool.tile([P, T], f32, name="qi", tag="qi")
            stt2(nc.vector, qr[:], pr[:], -1.0, rec[:], op.mult, op.mult)
            nc.vector.tensor_tensor(qi[:], pi, rec[:], op.mult)

            # diag shift: dre_i -= AG_ii (in place, ACT)
            for i in range(1, N):
                nc.scalar.activation(dre(i), dre(i), AF.Copy, bias=-AGf[i][i])

            # M_ij^(1) = -AG_ij + Pq_ij*q (+ d_i on diag); RHS: B_i - PB_i*q
            for i in range(1, N):
                e = eng(i)
                for j in range(1, N):
                    if j == i:
                        stt2(e, MR(i, i), qr[:], Pq[i][i], dre(i), op.mult, op.add)
                        stt2(e, MI(i, i), qi[:], Pq[i][i], dim(i), op.mult, op.add)
                    else:
                        h = (i * 8 + j) % 5
                        if h < 2:
                            nc.scalar.activation(MR(i, j), qr[:], AF.Copy,
                                                 scale=Pq[i][j], bias=-AGf[i][j])
                            nc.scalar.activation(MI(i, j), qi[:], AF.Copy, scale=Pq[i][j])
                        else:
                            e2 = nc.vector if h < 4 else nc.gpsimd
                            e2.tensor_scalar(MR(i, j), qr[:], Pq[i][j], -AGf[i][j],
                                             op.mult, op.add)
                            e2.tensor_scalar(MI(i, j), qi[:], Pq[i][j], None, op.mult)
                nc.vector.tensor_scalar(MR(i, 8), qr[:], -PB[i], Bf[i], op.mult, op.add)
                nc.scalar.activation(MI(i, 8), qi[:], AF.Copy, scale=-PB[i])

            # persistent pivot inverses (reused in back-substitution)
            inv = [None] * N

            def make_inv(k):
                deni = pool.tile([P, T], f32, name="deni", tag="den", bufs=2)
                mm = pool.tile([P, T], f32, name="mm", tag="m1", bufs=2)
                nc.vector.tensor_tensor(deni[:], MR(k, k), MR(k, k), op.mult)
                nc.vector.tensor_tensor(mm[:], MI(k, k), MI(k, k), op.mult)
                nc.vector.tensor_tensor(deni[:], deni[:], mm[:], op.add)
                reci = pool.tile([P, T], f32, name="reci", tag="rec", bufs=2)
                nc.vector.reciprocal(reci[:], deni[:])
                inv[k] = pool.tile([P, T2], f32, name=f"inv{k}", tag=f"inv{k}")
                nc.vector.tensor_tensor(inv[k][:, :T], MR(k, k), reci[:], op.mult)
                stt2(nc.vector, inv[k][:, T:], MI(k, k), -1.0,
                     reci[:], op.mult, op.mult)

            # ---- elimination steps k=1..6 ----
            CMAX = cmax  # wide-span cap (SBUF scratch sizing)
            make_inv(1)
            pvA = {}
            pvB = {}
            for k in range(1, 7):
                if karatsuba:
                    call = 8 - k
                    i_pv = k
                    sk4 = MP(k, k + 1, call).rearrange("p (c two t) -> p c two t",
                                                       two=2, t=T)
                    pvA[k] = pool.tile([P, call * T], f32, name="pvA", tag="pvA", bufs=2)
                    pvB[k] = pool.tile([P, call * T], f32, name="pvB", tag="pvB", bufs=2)
                    nc.vector.tensor_tensor(
                        pvA[k][:].rearrange("p (c t) -> p c t", c=call),
                        sk4[:, :, 1, :], sk4[:, :, 0, :], op.subtract)
                    nc.gpsimd.tensor_tensor(
                        pvB[k][:].rearrange("p (c t) -> p c t", c=call),
                        sk4[:, :, 0, :], sk4[:, :, 1, :], op.add)
                for i in range(k + 1, N):
                    e = eng(i)
                    et = nc.vector if tchain_dve else e
                    # t_i = M_ik * inv_k  (complex, packed into tpk)
                    tpk = pool.tile([P, T2], f32, name="tpk", tag="tpk", bufs=abufs)
                    w1 = pool.tile([P, T2], f32, name="w1", tag="w1", bufs=w12bufs)
                    w2 = pool.tile([P, T2], f32, name="w2", tag="w2", bufs=w12bufs)
                    iv = inv[k][:]
                    ivs = iv.rearrange("p (two t) -> p two t", two=2, t=T)[:, ::-1, :]
                    et.tensor_tensor(w1[:], MP(i, k), iv, op.mult)
                    et.tensor_tensor(w2[:].rearrange("p (two t) -> p two t", two=2, t=T),
                                    MP(i, k).rearrange("p (two t) -> p two t", two=2, t=T),
                                    ivs, op.mult)
                    et.tensor_tensor(tpk[:, :T], w1[:, :T], w1[:, T:], op.subtract)
                    et.tensor_tensor(tpk[:, T:], w2[:, :T], w2[:, T:], op.add)
                    tb3 = tpk[:].unsqueeze(1)
                    tb4 = tpk[:].rearrange("p (two t) -> p two t", two=2).unsqueeze(1)
                    if karatsuba:
                        # 3-mult complex update: k1=mr*(tr+ti), k2=tr*(mi-mr),
                        # k3=ti*(mr+mi); re -= k1-k3, im -= k1+k2
                        call = 8 - k
                        tsum = pool.tile([P, T], f32, name="tsum", tag="tsum", bufs=abufs)
                        e.tensor_tensor(tsum[:], tpk[:, :T], tpk[:, T:], op.add)
                        tsb = tsum[:].unsqueeze(1)
                        trb = tpk[:, :T].unsqueeze(1)
                        tib = tpk[:, T:].unsqueeze(1)
                        j = k + 1
                        while j <= 8:
                            c = min(CMAX, 8 - j + 1)
                            co = (j - k - 1) * T  # offset into pvA/pvB
                            si4 = MP(i, j, c).rearrange("p (c two t) -> p c two t",
                                                        two=2, t=T)
                            sk4 = MP(k, j, c).rearrange("p (c two t) -> p c two t",
                                                        two=2, t=T)
                            K1 = pool.tile([P, c * T], f32, name="K1", tag="K1", bufs=kbufs)
                            K2 = pool.tile([P, c * T], f32, name="K2", tag="K2", bufs=kbufs)
                            K3 = pool.tile([P, c * T], f32, name="K3", tag="K3", bufs=kbufs)
                            K13 = K1[:].rearrange("p (c t) -> p c t", c=c)
                            K23 = K2[:].rearrange("p (c t) -> p c t", c=c)
                            K33 = K3[:].rearrange("p (c t) -> p c t", c=c)
                            e.tensor_tensor(K13, sk4[:, :, 0, :],
                                            tsb.broadcast_to([P, c, T]), op.mult)
                            e.tensor_tensor(K23, pvA[k][:, co:co + c * T]
                                            .rearrange("p (c t) -> p c t", c=c),
                                            trb.broadcast_to([P, c, T]), op.mult)
                            e.tensor_tensor(K33, pvB[k][:, co:co + c * T]
                                            .rearrange("p (c t) -> p c t", c=c),
                                            tib.broadcast_to([P, c, T]), op.mult)
                            e.tensor_tensor(si4[:, :, 0, :], si4[:, :, 0, :], K13, op.subtract)
                            e.tensor_tensor(si4[:, :, 0, :], si4[:, :, 0, :], K33, op.add)
                            e.tensor_tensor(si4[:, :, 1, :], si4[:, :, 1, :], K13, op.subtract)
                            e.tensor_tensor(si4[:, :, 1, :], si4[:, :, 1, :], K23, op.subtract)
                            j += c
                        if i == k + 1:
                            make_inv(k + 1)
                        continue
                    # wide span updates, chunked to <= CMAX column blocks
                    j = k + 1
                    while j <= 8:
                        c = min(CMAX, 8 - j + 1)
                        si4 = MP(i, j, c).rearrange("p (c two t) -> p c two t",
                                                    two=2, t=T)
                        sk3 = MP(k, j, c).rearrange("p (c x) -> p c x", c=c)
                        sk4s = MP(k, j, c).rearrange("p (c two t) -> p c two t",
                                                    two=2, t=T)[:, :, ::-1, :]
                        wA = pool.tile([P, c * T2], f32, name="wA", tag="wA", bufs=wbufs)
                        wB = pool.tile([P, c * T2], f32, name="wB", tag="wB", bufs=wbufs)
                        wA3 = wA[:].rearrange("p (c x) -> p c x", c=c)
                        wA4 = wA[:].rearrange("p (c two t) -> p c two t", two=2, t=T)
                        wB4 = wB[:].rearrange("p (c two t) -> p c two t", two=2, t=T)
                        eB = nc.gpsimd if (split_paths and e is nc.vector
                                           and i % 2 == 0) or i in pool_wb else e
                        e.tensor_tensor(wA3, sk3, tb3.broadcast_to([P, c, T2]), op.mult)
                        eB.tensor_tensor(wB4, sk4s, tb4.broadcast_to([P, c, 2, T]), op.mult)
                        e.tensor_tensor(wA4[:, :, 0, :], wA4[:, :, 0, :],
                                        wA4[:, :, 1, :], op.subtract)
                        eB.tensor_tensor(wB4[:, :, 0, :], wB4[:, :, 0, :],
                                        wB4[:, :, 1, :], op.add)
                        e.tensor_tensor(si4[:, :, 0, :], si4[:, :, 0, :],
                                        wA4[:, :, 0, :], op.subtract)
                        eB.tensor_tensor(si4[:, :, 1, :], si4[:, :, 1, :],
                                        wB4[:, :, 0, :], op.subtract)
                        j += c
                    if i == k + 1:
                        # next pivot row is final -> immediately queue its
                        # inverse (critical chain for step k+1)
                        make_inv(k + 1)

            # ---- back-substitution ----
            y = pool.tile([P, 8 * T2], f32, name="y", tag="dy")  # reuses d's slot

            def yP(i):
                return y[:, i * T2:(i + 1) * T2]

            def yR(i):
                return y[:, i * T2:i * T2 + T]

            def yI(i):
                return y[:, i * T2 + T:(i + 1) * T2]

            def divide(i):
                # y_i = M_i8 * inv_i  (packed: 4 ops)
                b1 = pool.tile([P, T2], f32, name="b1", tag="w1", bufs=w12bufs)
                b2 = pool.tile([P, T2], f32, name="b2", tag="w2", bufs=w12bufs)
                iv = inv[i][:]
                ivs = iv.rearrange("p (two t) -> p two t", two=2, t=T)[:, ::-1, :]
                nc.vector.tensor_tensor(b1[:], MP(i, 8), iv, op.mult)
                nc.vector.tensor_tensor(b2[:].rearrange("p (two t) -> p two t", two=2, t=T),
                                        MP(i, 8).rearrange("p (two t) -> p two t", two=2, t=T),
                                        ivs, op.mult)
                nc.vector.tensor_tensor(yR(i), b1[:, :T], b1[:, T:], op.subtract)
                nc.vector.tensor_tensor(yI(i), b2[:, :T], b2[:, T:], op.add)

            def bterm(i, jj):
                e = eng(i)
                # M_i8 -= M_i,jj * y_jj
                pA = pool.tile([P, T2], f32, name="pA", tag="pA", bufs=abufs)
                pB = pool.tile([P, T2], f32, name="pB", tag="pB", bufs=abufs)
                e.tensor_tensor(pA[:], MP(i, jj), yP(jj), op.mult)
                Msw = MP(i, jj).rearrange("p (two t) -> p two t", two=2)[:, ::-1, :]
                e.tensor_tensor(pB[:].rearrange("p (two t) -> p two t", two=2),
                                Msw, yP(jj).rearrange("p (two t) -> p two t", two=2),
                                op.mult)
                e.tensor_tensor(pA[:, :T], pA[:, :T], pA[:, T:], op.subtract)
                e.tensor_tensor(pA[:, T:], pB[:, :T], pB[:, T:], op.add)
                e.tensor_tensor(MP(i, 8), MP(i, 8), pA[:], op.subtract)

            divide(7)
            for jj in range(7, 1, -1):
                bterm(jj - 1, jj)   # gates the next divide: emit first
                divide(jj - 1)
                for i in range(jj - 2, 0, -1):
                    bterm(i, jj)

            # y_0 = -q * (B_0 + sum_j AG_0j * y_j)
            acc0 = pool.tile([P, T2], f32, name="acc0", tag="acc0")
            nc.scalar.activation(acc0[:, :T], yR(1), AF.Copy, scale=AGf[0][1], bias=Bf[0])
            nc.scalar.activation(acc0[:, T:], yI(1), AF.Copy, scale=AGf[0][1])
            for j in range(2, N):
                stt2(nc.vector, acc0[:], yP(j), AGf[0][j], acc0[:],
                     op.mult, op.add)
            c1 = pool.tile([P, T], f32, name="c1", tag="w1", bufs=w12bufs)
            c2 = pool.tile([P, T], f32, name="c2", tag="w2", bufs=w12bufs)
            nc.vector.tensor_tensor(c1[:], qi[:], acc0[:, T:], op.mult)
            nc.vector.tensor_tensor(c2[:], qr[:], acc0[:, :T], op.mult)
            nc.vector.tensor_tensor(yR(0), c1[:], c2[:], op.subtract)
            nc.vector.tensor_tensor(c1[:], qr[:], acc0[:, T:], op.mult)
            nc.vector.tensor_tensor(c2[:], qi[:], acc0[:, :T], op.mult)
            nc.vector.tensor_tensor(c1[:], c1[:], c2[:], op.add)
            nc.vector.tensor_scalar(yI(0), c1[:], -1.0, None, op.mult)

            # ---- outputs: H_i = C_i * y_i ; S = sum_i H_i ----
            Hout = pool.tile([P, 16 * T], f32, name="Hout", tag="io")  # reuses theta's slot

            def HP(i):
                return Hout[:, i * T2:(i + 1) * T2]

            for i in range(N):
                e3 = nc.gpsimd if i in (2, 5) else (
                    nc.vector if i % 2 == 0 else nc.vector)
                e3.tensor_scalar(HP(i), yP(i), Cf[i], None, op.mult)
            s01 = pool.tile([P, T2], f32, name="s01", tag="s01")
            s23 = pool.tile([P, T2], f32, name="s23", tag="s23")
            s45 = pool.tile([P, T2], f32, name="s45", tag="s45")
            s67 = pool.tile([P, T2], f32, name="s67", tag="s67")
            nc.vector.tensor_tensor(s01[:], HP(0), HP(1), op.add)
            nc.gpsimd.tensor_tensor(s23[:], HP(2), HP(3), op.add)
            nc.vector.tensor_tensor(s45[:], HP(4), HP(5), op.add)
            nc.gpsimd.tensor_tensor(s67[:], HP(6), HP(7), op.add)
            nc.vector.tensor_tensor(s01[:], s01[:], s23[:], op.add)
            nc.vector.tensor_tensor(s45[:], s45[:], s67[:], op.add)
            Sout = pool.tile([P, T2], f32, name="Sout", tag="Sout")
            nc.vector.tensor_tensor(Sout[:], s01[:], s45[:], op.add)

            nc.sync.dma_start(H_d[:], Hout[:])
            nc.sync.dma_start(S_d[:], Sout[:])

    nc.compile()
    return nc


def kernel(x, B, C, W, m):
    from concourse.bass_utils import run_bass_kernel_spmd

    B = np.asarray(B)
    C = np.asarray(C)
    W = np.asarray(W)
    m = np.asarray(m)

    md, AG = _host_params(B, C, W, m)
    theta = _theta_planes(x, md)  # [16, FPAD] f32

    # per-core input: [128, 16*T] with plane-major blocks, f_local = p*T + t
    in_maps = []
    for c in range(NCORES):
        chunk = theta[:, c * FC:(c + 1) * FC].reshape(16, P, T)
        in_maps.append({"theta": np.ascontiguousarray(
            chunk.transpose(1, 0, 2).reshape(P, 16 * T))})

    nc = _build_nc(AG, B.reshape(-1), C.reshape(-1))
    res = run_bass_kernel_spmd(nc, in_maps, list(range(NCORES)))

    H = np.empty((FPAD, N), dtype=np.complex64)
    Hs = np.empty(FPAD, dtype=np.complex64)
    for c in range(NCORES):
        o = res.results[c]["H"].reshape(P, 16, T)
        s = res.results[c]["S"].reshape(P, 2, T)
        sl = slice(c * FC, (c + 1) * FC)
        for i in range(N):
            H[sl, i] = (o[:, 2 * i, :] + 1j * o[:, 2 * i + 1, :]).reshape(FC)
        Hs[sl] = (s[:, 0, :] + 1j * s[:, 1, :]).reshape(FC)
    H = H[:F]
    Hs = Hs[:F]

    h = np.fft.irfft(Hs.astype(np.complex128), n=NFFT)
    h = (h / np.max(np.abs(h))).astype(np.float32)
    return H.astype(np.complex64), h


# revision 21
# speedup vs baseline: 1.0392x; 1.0377x over previous
"""Trainium2 Bass kernel for nn_DiffFDN: H(z) = C^T (diag(z^m) - A*Gamma)^-1 B
evaluated on F=192001 frequencies, plus h = normalized irfft of sum(H*C).

Strategy (per sharding hint): shard the frequency axis across 8 NeuronCores
(pure data parallel, 24064 freqs/core laid out as [128 partitions x 188 cols]).
Each core evaluates the per-frequency diagonal d_i = z^{m_i} with ACT Sin
(host supplies exactly range-reduced phase arguments via integer arithmetic)
and solves the 8x8 complex system per frequency with a fully unrolled,
unpivoted augmented Gaussian elimination on the vector engines (safe: the
leading principal minors of D - A*Gamma are bounded away from 0 because
||A*Gamma||_2 <= max(gamma) < 1 and |d_i| = 1).

Host only does O(N)/O(F) integer bookkeeping, the tiny 8x8 expm, and the
final irfft + normalization of the gathered F-vector.
"""

import numpy as np

# ---- module constants of the reference nn.Module (not inputs) ----
_DELAYS = np.array([809., 877., 937., 1049., 1151., 1249., 1373., 1499.],
                   dtype=np.float32)
N = 8
F = 192001
NFFT = 384000
HALF = 192000           # NFFT // 2
GAIN_PER_SAMPLE = 0.9999
NCORES = 8
P = 128                 # SBUF partitions
T = 188                 # free-dim columns per plane
FC = P * T              # 24064 freqs per core
FPAD = FC * NCORES      # 192512

_PI_SAFE = float(np.nextafter(np.float32(np.pi), np.float32(0.0)))


def _host_params(B, C, W, m):
    """Tiny N=8 parameter prep (mirrors the reference's fp32 arithmetic)."""
    M_AVR = _DELAYS.mean(dtype=np.float32)
    M_STD = _DELAYS.std(ddof=1, dtype=np.float32)
    md = (m.astype(np.float32) * np.float32(M_STD) + np.float32(M_AVR)).astype(np.float32)
    gamma = np.power(np.float32(GAIN_PER_SAMPLE), md).astype(np.float32)
    # A = expm(S - S^T) via eigendecomposition of the Hermitian i*(S - S^T)
    S = np.triu(W.astype(np.float64), 1)
    K = S - S.T
    lam, V = np.linalg.eigh(1j * K)
    A = (V @ np.diag(np.exp(-1j * lam)) @ V.conj().T).real.astype(np.float32)
    AG = (A * gamma[None, :]).astype(np.float32)
    return md, AG


def _theta_planes(x, md):
    """[16, FPAD] float32 phase planes, exactly reduced to [-pi, pi].

    Plane 2i   -> argument of cos(md_i * w)  (as sin(arg + pi/2))
    Plane 2i+1 -> argument of sin(md_i * w)
    """
    md64 = md.astype(np.float64)
    m_int = np.round(md64).astype(np.int64)
    res = md64 - m_int

    f_idx = np.arange(FPAD, dtype=np.int64)
    f_idx[F:] = HALF  # padding tail: any valid frequency

    # Verify x matches the canonical grid exp(i*pi*f/192000); if not, fall
    # back to computing phases from angle(x) directly (still exact reduction).
    grid_ok = False
    xc = np.asarray(x)
    if xc.shape == (F,) and np.iscomplexobj(xc):
        wg = np.pi * np.arange(F, dtype=np.float64) / HALF
        dev = np.max(np.abs(xc.astype(np.complex128) - np.exp(1j * wg)))
        grid_ok = bool(dev < 1e-4)

    theta = np.empty((16, FPAD), dtype=np.float64)
    if grid_ok:
        t_sin = (f_idx[None, :] * m_int[:, None] + HALF) % NFFT - HALF
        t_cos = (f_idx[None, :] * m_int[:, None] + 96000 + HALF) % NFFT - HALF
        theta[0::2] = t_cos * (np.pi / HALF)
        theta[1::2] = t_sin * (np.pi / HALF)
        if np.any(res != 0.0):
            w = (np.pi / HALF) * f_idx.astype(np.float64)
            theta[0::2] += w[None, :] * res[:, None]
            theta[1::2] += w[None, :] * res[:, None]
            theta = (theta + np.pi) % (2 * np.pi) - np.pi
    else:
        w = np.angle(xc.astype(np.complex128))
        w = np.concatenate([w, np.full(FPAD - F, w[-1])])
        base = md64[:, None] * w[None, :]
        theta[0::2] = (base + np.pi / 2 + np.pi) % (2 * np.pi) - np.pi
        theta[1::2] = (base + np.pi) % (2 * np.pi) - np.pi

    return np.clip(theta, -_PI_SAFE, _PI_SAFE).astype(np.float32)


def _build_nc(AG, Bv, Cv, pool_rows=(3, 6), cmax=4, wbufs=3, abufs=4, w12bufs=3, split_paths=False,
               pool_wb=(), tchain_dve=False, karatsuba=False, kbufs=2):
    """Build the single-core Bass/Tile program (SPMD across 8 cores)."""
    import concourse.bacc as bacc
    import concourse.bass as bass
    import concourse.mybir as mybir
    import concourse.tile as tile

    f32 = mybir.dt.float32
    AF = mybir.ActivationFunctionType
    op = mybir.AluOpType
    T2 = 2 * T

    AGf = [[float(AG[i, j]) for j in range(N)] for i in range(N)]
    Bf = [float(Bv[i]) for i in range(N)]
    Cf = [float(Cv[i]) for i in range(N)]
    # step-0 fold constants: l_i = AG[i,0]*q with q = -1/(d0 - AG00)
    Pq = [[AGf[i][0] * AGf[0][j] for j in range(N)] for i in range(N)]
    PB = [AGf[i][0] * Bf[0] for i in range(N)]

    nc = bacc.Bacc(None)
    th_d = nc.dram_tensor("theta", [P, 16 * T], f32, kind="ExternalInput")
    H_d = nc.dram_tensor("H", [P, 16 * T], f32, kind="ExternalOutput")
    S_d = nc.dram_tensor("S", [P, T2], f32, kind="ExternalOutput")

    def eng(i):
        # split the independent per-row work across DVE and GPSIMD
        return nc.gpsimd if i in pool_rows else nc.vector

    with tile.TileContext(nc) as tc:
        with tc.tile_pool(name="main", bufs=1) as pool:
            def stt2(e, out, in0, scalar, in1, op0, op1):
                # (in0 op0 scalar) op1 in1 without the STT ISA struct
                # (walrus rejects STT when Tile needs >1 sync wait on it)
                w = out.shape[-1]
                tmp = pool.tile([P, w], f32, name="sttmp", tag=f"sttmp{w}", bufs=abufs)
                e.tensor_scalar(tmp[:], in0, scalar, None, op0)
                e.tensor_tensor(out, tmp[:], in1, op1)

            th = pool.tile([P, 16 * T], f32, name="th", tag="io")
            # blocks 0,1 land first so the step-0 pivot chain starts early
            nc.sync.dma_start(th[:, 0:2 * T], th_d[:, 0:2 * T])
            nc.sync.dma_start(th[:, 2 * T:16 * T], th_d[:, 2 * T:16 * T])

            # d planes: block 2i = cos_i (re), 2i+1 = sin_i (im)
            d = pool.tile([P, 16 * T], f32, name="d", tag="dy")
            # blocks 0,1 first (they gate the step-0 pivot chain), rest in one op
            nc.scalar.activation(d[:, 0:2 * T], th[:, 0:2 * T], AF.Sin)
            nc.scalar.activation(d[:, 2 * T:16 * T], th[:, 2 * T:16 * T], AF.Sin)

            def dre(i):
                return d[:, (2 * i) * T:(2 * i + 1) * T]

            def dim(i):
                return d[:, (2 * i + 1) * T:(2 * i + 2) * T]

            # augmented rows i=1..7, cols j=1..8 (8 = RHS), packed re|im
            Mrow = [None] + [pool.tile([P, 8 * T2], f32, name=f"Mrow{i}", tag=f"M{i}")
                             for i in range(1, N)]

            def MR(i, j):
                return Mrow[i][:, (j - 1) * T2:(j - 1) * T2 + T]

            def MI(i, j):
                return Mrow[i][:, (j - 1) * T2 + T:j * T2]

            def MP(i, j, c=1):
                return Mrow[i][:, (j - 1) * T2:(j - 1 + c) * T2]

            # ---- step 0 (pivot row 0 is constant: M0j = -AG0j, b0 = B0) ----
            pr = pool.tile([P, T], f32, name="pr", tag="pr")
            nc.vector.tensor_scalar(pr[:], dre(0), AGf[0][0], None, op.subtract)
            pi = dim(0)
            den = pool.tile([P, T], f32, name="den", tag="den", bufs=2)
            m1 = pool.tile([P, T], f32, name="m1", tag="m1", bufs=2)
            nc.vector.tensor_tensor(den[:], pr[:], pr[:], op.mult)
            nc.vector.tensor_tensor(m1[:], pi, pi, op.mult)
            nc.vector.tensor_tensor(den[:], den[:], m1[:], op.add)
            rec = pool.tile([P, T], f32, name="rec", tag="rec", bufs=2)
            nc.vector.reciprocal(rec[:], den[:])
            qr = pool.tile([P, T], f32, name="qr", tag="qr")
            qi = pool.tile([P, T], f32, name="qi", tag="qi")
            stt2(nc.vector, qr[:], pr[:], -1.0, rec[:], op.mult, op.mult)
            nc.vector.tensor_tensor(qi[:], pi, rec[:], op.mult)


            # M_ij^(1) = -AG_ij + Pq_ij*q (+ d_i on diag); RHS: B_i - PB_i*q
            for i in range(1, N):
                e = eng(i)
                for j in range(1, N):
                    if j == i:
                        # M_ii = (Pq_ii*qr - AG_ii) + d_i  (two-scalar ts + tt)
                        tmp = pool.tile([P, T], f32, name="dtmp", tag="sttmp188",
                                        bufs=abufs)
                        e.tensor_scalar(tmp[:], qr[:], Pq[i][i], AGf[i][i],
                                        op.mult, op.subtract)
                        e.tensor_tensor(MR(i, i), tmp[:], dre(i), op.add)
                        stt2(e, MI(i, i), qi[:], Pq[i][i], dim(i), op.mult, op.add)
                    else:
                        h = (i * 8 + j) % 5
                        if h < 2:
                            nc.scalar.activation(MR(i, j), qr[:], AF.Copy,
                                                 scale=Pq[i][j], bias=-AGf[i][j])
                            nc.scalar.activation(MI(i, j), qi[:], AF.Copy, scale=Pq[i][j])
                        else:
                            e2 = nc.vector if h < 4 else nc.gpsimd
                            e2.tensor_scalar(MR(i, j), qr[:], Pq[i][j], -AGf[i][j],
                                             op.mult, op.add)
                            e2.tensor_scalar(MI(i, j), qi[:], Pq[i][j], None, op.mult)
                nc.vector.tensor_scalar(MR(i, 8), qr[:], -PB[i], Bf[i], op.mult, op.add)
                nc.scalar.activation(MI(i, 8), qi[:], AF.Copy, scale=-PB[i])

            # persistent pivot inverses (reused in back-substitution)
            inv = [None] * N

            def make_inv(k):
                deni = pool.tile([P, T], f32, name="deni", tag="den", bufs=2)
                mm = pool.tile([P, T], f32, name="mm", tag="m1", bufs=2)
                nc.vector.tensor_tensor(deni[:], MR(k, k), MR(k, k), op.mult)
                nc.vector.tensor_tensor(mm[:], MI(k, k), MI(k, k), op.mult)
                nc.vector.tensor_tensor(deni[:], deni[:], mm[:], op.add)
                reci = pool.tile([P, T], f32, name="reci", tag="rec", bufs=2)
                nc.vector.reciprocal(reci[:], deni[:])
                inv[k] = pool.tile([P, T2], f32, name=f"inv{k}", tag=f"inv{k}")
                nc.vector.tensor_tensor(inv[k][:, :T], MR(k, k), reci[:], op.mult)
                stt2(nc.vector, inv[k][:, T:], MI(k, k), -1.0,
                     reci[:], op.mult, op.mult)

            # ---- elimination steps k=1..6 ----
            CMAX = cmax  # wide-span cap (SBUF scratch sizing)
            make_inv(1)
            pvA = {}
            pvB = {}
            for k in range(1, 7):
                if karatsuba:
                    call = 8 - k
                    i_pv = k
                    sk4 = MP(k, k + 1, call).rearrange("p (c two t) -> p c two t",
                                                       two=2, t=T)
                    pvA[k] = pool.tile([P, call * T], f32, name="pvA", tag="pvA", bufs=2)
                    pvB[k] = pool.tile([P, call * T], f32, name="pvB", tag="pvB", bufs=2)
                    nc.vector.tensor_tensor(
                        pvA[k][:].rearrange("p (c t) -> p c t", c=call),
                        sk4[:, :, 1, :], sk4[:, :, 0, :], op.subtract)
                    nc.gpsimd.tensor_tensor(
                        pvB[k][:].rearrange("p (c t) -> p c t", c=call),
                        sk4[:, :, 0, :], sk4[:, :, 1, :], op.add)
                for i in range(k + 1, N):
                    e = eng(i)
                    et = nc.vector if tchain_dve else e
                    # the i==k+1 chain gates the next pivot: split its re/im
                    # paths across engines to halve serial depth
                    gate = False  # splitting the gate row onto Pool measured slower
                    etB = et
                    # t_i = M_ik * inv_k  (complex, packed into tpk)
                    tpk = pool.tile([P, T2], f32, name="tpk", tag="tpk", bufs=abufs)
                    w1 = pool.tile([P, T2], f32, name="w1", tag="w1", bufs=w12bufs)
                    w2 = pool.tile([P, T2], f32, name="w2", tag="w2", bufs=w12bufs)
                    iv = inv[k][:]
                    ivs = iv.rearrange("p (two t) -> p two t", two=2, t=T)[:, ::-1, :]
                    et.tensor_tensor(w1[:], MP(i, k), iv, op.mult)
                    etB.tensor_tensor(w2[:].rearrange("p (two t) -> p two t", two=2, t=T),
                                    MP(i, k).rearrange("p (two t) -> p two t", two=2, t=T),
                                    ivs, op.mult)
                    et.tensor_tensor(tpk[:, :T], w1[:, :T], w1[:, T:], op.subtract)
                    etB.tensor_tensor(tpk[:, T:], w2[:, :T], w2[:, T:], op.add)
                    tb3 = tpk[:].unsqueeze(1)
                    tb4 = tpk[:].rearrange("p (two t) -> p two t", two=2).unsqueeze(1)
                    if karatsuba:
                        # 3-mult complex update: k1=mr*(tr+ti), k2=tr*(mi-mr),
                        # k3=ti*(mr+mi); re -= k1-k3, im -= k1+k2
                        call = 8 - k
                        tsum = pool.tile([P, T], f32, name="tsum", tag="tsum", bufs=abufs)
                        e.tensor_tensor(tsum[:], tpk[:, :T], tpk[:, T:], op.add)
                        tsb = tsum[:].unsqueeze(1)
                        trb = tpk[:, :T].unsqueeze(1)
                        tib = tpk[:, T:].unsqueeze(1)
                        j = k + 1
                        while j <= 8:
                            c = min(CMAX, 8 - j + 1)
                            co = (j - k - 1) * T  # offset into pvA/pvB
                            si4 = MP(i, j, c).rearrange("p (c two t) -> p c two t",
                                                        two=2, t=T)
                            sk4 = MP(k, j, c).rearrange("p (c two t) -> p c two t",
                                                        two=2, t=T)
                            K1 = pool.tile([P, c * T], f32, name="K1", tag="K1", bufs=kbufs)
                            K2 = pool.tile([P, c * T], f32, name="K2", tag="K2", bufs=kbufs)
                            K3 = pool.tile([P, c * T], f32, name="K3", tag="K3", bufs=kbufs)
                            K13 = K1[:].rearrange("p (c t) -> p c t", c=c)
                            K23 = K2[:].rearrange("p (c t) -> p c t", c=c)
                            K33 = K3[:].rearrange("p (c t) -> p c t", c=c)
                            e.tensor_tensor(K13, sk4[:, :, 0, :],
                                            tsb.broadcast_to([P, c, T]), op.mult)
                            e.tensor_tensor(K23, pvA[k][:, co:co + c * T]
                                            .rearrange("p (c t) -> p c t", c=c),
                                            trb.broadcast_to([P, c, T]), op.mult)
                            e.tensor_tensor(K33, pvB[k][:, co:co + c * T]
                                            .rearrange("p (c t) -> p c t", c=c),
                                            tib.broadcast_to([P, c, T]), op.mult)
                            e.tensor_tensor(si4[:, :, 0, :], si4[:, :, 0, :], K13, op.subtract)
                            e.tensor_tensor(si4[:, :, 0, :], si4[:, :, 0, :], K33, op.add)
                            e.tensor_tensor(si4[:, :, 1, :], si4[:, :, 1, :], K13, op.subtract)
                            e.tensor_tensor(si4[:, :, 1, :], si4[:, :, 1, :], K23, op.subtract)
                            j += c
                        if i == k + 1:
                            make_inv(k + 1)
                        continue
                    # wide span updates, chunked to <= CMAX column blocks
                    j = k + 1
                    while j <= 8:
                        c = min(CMAX, 8 - j + 1)
                        si4 = MP(i, j, c).rearrange("p (c two t) -> p c two t",
                                                    two=2, t=T)
                        sk3 = MP(k, j, c).rearrange("p (c x) -> p c x", c=c)
                        sk4s = MP(k, j, c).rearrange("p (c two t) -> p c two t",
                                                    two=2, t=T)[:, :, ::-1, :]
                        wA = pool.tile([P, c * T2], f32, name="wA", tag="wA", bufs=wbufs)
                        wB = pool.tile([P, c * T2], f32, name="wB", tag="wB", bufs=wbufs)
                        wA3 = wA[:].rearrange("p (c x) -> p c x", c=c)
                        wA4 = wA[:].rearrange("p (c two t) -> p c two t", two=2, t=T)
                        wB4 = wB[:].rearrange("p (c two t) -> p c two t", two=2, t=T)
                        eB = nc.gpsimd if (split_paths and e is nc.vector
                                           and i % 2 == 0) or i in pool_wb else (
                            etB if gate else e)
                        e.tensor_tensor(wA3, sk3, tb3.broadcast_to([P, c, T2]), op.mult)
                        eB.tensor_tensor(wB4, sk4s, tb4.broadcast_to([P, c, 2, T]), op.mult)
                        e.tensor_tensor(wA4[:, :, 0, :], wA4[:, :, 0, :],
                                        wA4[:, :, 1, :], op.subtract)
                        eB.tensor_tensor(wB4[:, :, 0, :], wB4[:, :, 0, :],
                                        wB4[:, :, 1, :], op.add)
                        e.tensor_tensor(si4[:, :, 0, :], si4[:, :, 0, :],
                                        wA4[:, :, 0, :], op.subtract)
                        eB.tensor_tensor(si4[:, :, 1, :], si4[:, :, 1, :],
                                        wB4[:, :, 0, :], op.subtract)
                        j += c
                    if i == k + 1:
                        # next pivot row is final -> immediately queue its
                        # inverse (critical chain for step k+1)
                        make_inv(k + 1)

            # ---- back-substitution ----
            y = pool.tile([P, 8 * T2], f32, name="y", tag="dy")  # reuses d's slot

            def yP(i):
                return y[:, i * T2:(i + 1) * T2]

            def yR(i):
                return y[:, i * T2:i * T2 + T]

            def yI(i):
                return y[:, i * T2 + T:(i + 1) * T2]

            def divide(i):
                # y_i = M_i8 * inv_i  (packed; re path on DVE, im on Pool)
                b1 = pool.tile([P, T2], f32, name="b1", tag="w1", bufs=w12bufs)
                b2 = pool.tile([P, T2], f32, name="b2", tag="w2", bufs=w12bufs)
                iv = inv[i][:]
                ivs = iv.rearrange("p (two t) -> p two t", two=2, t=T)[:, ::-1, :]
                nc.vector.tensor_tensor(b1[:], MP(i, 8), iv, op.mult)
                nc.gpsimd.tensor_tensor(b2[:].rearrange("p (two t) -> p two t", two=2, t=T),
                                        MP(i, 8).rearrange("p (two t) -> p two t", two=2, t=T),
                                        ivs, op.mult)
                nc.vector.tensor_tensor(yR(i), b1[:, :T], b1[:, T:], op.subtract)
                nc.gpsimd.tensor_tensor(yI(i), b2[:, :T], b2[:, T:], op.add)

            def bterm(i, jj, split=False):
                # M_i8 -= M_i,jj * y_jj; split=True halves the serial depth
                # by running the im-product path on Pool (used on the chain
                # that gates the next divide)
                e = eng(i)
                eB = nc.gpsimd if split and e is nc.vector else e
                pA = pool.tile([P, T2], f32, name="pA", tag="pA", bufs=abufs)
                pB = pool.tile([P, T2], f32, name="pB", tag="pB", bufs=abufs)
                e.tensor_tensor(pA[:], MP(i, jj), yP(jj), op.mult)
                Msw = MP(i, jj).rearrange("p (two t) -> p two t", two=2)[:, ::-1, :]
                eB.tensor_tensor(pB[:].rearrange("p (two t) -> p two t", two=2),
                                 Msw, yP(jj).rearrange("p (two t) -> p two t", two=2),
                                 op.mult)
                e.tensor_tensor(pA[:, :T], pA[:, :T], pA[:, T:], op.subtract)
                eB.tensor_tensor(pA[:, T:], pB[:, :T], pB[:, T:], op.add)
                e.tensor_tensor(MP(i, 8), MP(i, 8), pA[:], op.subtract)

            Hout = pool.tile([P, 16 * T], f32, name="Hout", tag="io")

            def HP(i):
                return Hout[:, i * T2:(i + 1) * T2]

            def emit_H(i):
                # H_i = C_i*y_i: scale on ACT, stream the block to DRAM
                nc.scalar.activation(HP(i), yP(i), AF.Copy, scale=Cf[i])
                nc.sync.dma_start(H_d[:, i * T2:(i + 1) * T2], HP(i))

            # acc0 terms (AG_0j * y_j) and Hsum pairs, emitted as y_j lands
            a0 = {}
            SH = {}
            exA = {}
            exS = {}

            def post_y(i):
                emit_H(i)
                if i >= 1:
                    a0[i] = pool.tile([P, T2], f32, name=f"a0_{i}",
                                      tag="a0o" if i % 2 else "a0e", bufs=2)
                    if i == 1:
                        nc.vector.tensor_scalar(a0[1][:, :T], yR(1), AGf[0][1],
                                                Bf[0], op.mult, op.add)
                        nc.vector.tensor_scalar(a0[1][:, T:], yI(1), AGf[0][1],
                                                None, op.mult)
                    else:
                        nc.vector.tensor_scalar(a0[i][:], yP(i), AGf[0][i],
                                                None, op.mult)
                if i in (6, 4, 2):
                    exA[i] = pool.tile([P, T2], f32, name=f"exA{i}",
                                       tag="exA6" if i == 6 else "exAt", bufs=1 if i == 6 else 2)
                    nc.vector.tensor_tensor(exA[i][:], a0[i + 1][:], a0[i][:], op.add)
                    exS[i] = pool.tile([P, T2], f32, name=f"exS{i}",
                                       tag="exS6" if i == 6 else "exSt", bufs=1 if i == 6 else 2)
                    nc.gpsimd.tensor_tensor(exS[i][:], HP(i + 1), HP(i), op.add)
                if i == 4:
                    nc.vector.tensor_tensor(exA[6][:], exA[6][:], exA[4][:], op.add)
                    nc.gpsimd.tensor_tensor(exS[6][:], exS[6][:], exS[4][:], op.add)
                if i == 2:
                    nc.vector.tensor_tensor(exA[6][:], exA[6][:], exA[2][:], op.add)
                    nc.gpsimd.tensor_tensor(exS[6][:], exS[6][:], exS[2][:], op.add)

            divide(7)
            post_y(7)
            for jj in range(7, 1, -1):
                bterm(jj - 1, jj, split=True)  # gates the next divide
                divide(jj - 1)
                post_y(jj - 1)
                for i in range(jj - 2, 0, -1):
                    bterm(i, jj)

            # y_0 = -q * acc0, acc0 = B_0 + sum_j AG_0j*y_j (tree built above)
            acc0 = pool.tile([P, T2], f32, name="acc0", tag="acc0")
            nc.vector.tensor_tensor(acc0[:], exA[6][:], a0[1][:], op.add)
            c1 = pool.tile([P, T], f32, name="c1", tag="w1", bufs=w12bufs)
            c2 = pool.tile([P, T], f32, name="c2", tag="w2", bufs=w12bufs)
            nc.vector.tensor_tensor(c1[:], qi[:], acc0[:, T:], op.mult)
            nc.vector.tensor_tensor(c2[:], qr[:], acc0[:, :T], op.mult)
            nc.vector.tensor_tensor(yR(0), c1[:], c2[:], op.subtract)
            nc.vector.tensor_tensor(c1[:], qr[:], acc0[:, T:], op.mult)
            nc.vector.tensor_tensor(c2[:], qi[:], acc0[:, :T], op.mult)
            nc.vector.tensor_tensor(c1[:], c1[:], c2[:], op.add)
            nc.vector.tensor_scalar(yI(0), c1[:], -1.0, None, op.mult)

            # ---- Hsum: fold the last pair into the arrival-ordered tree ----
            emit_H(0)
            s01 = pool.tile([P, T2], f32, name="s01", tag="s01")
            nc.vector.tensor_tensor(s01[:], HP(0), HP(1), op.add)
            Sout = pool.tile([P, T2], f32, name="Sout", tag="Sout")
            nc.vector.tensor_tensor(Sout[:], exS[6][:], s01[:], op.add)

            nc.sync.dma_start(S_d[:], Sout[:])

    nc.compile()
    return nc


def kernel(x, B, C, W, m):
    from concourse.bass_utils import run_bass_kernel_spmd

    B = np.asarray(B)
    C = np.asarray(C)
    W = np.asarray(W)
    m = np.asarray(m)

    md, AG = _host_params(B, C, W, m)
    theta = _theta_planes(x, md)  # [16, FPAD] f32

    # per-core input: [128, 16*T] with plane-major blocks, f_local = p*T + t
    in_maps = []
    for c in range(NCORES):
        chunk = theta[:, c * FC:(c + 1) * FC].reshape(16, P, T)
        in_maps.append({"theta": np.ascontiguousarray(
            chunk.transpose(1, 0, 2).reshape(P, 16 * T))})

    nc = _build_nc(AG, B.reshape(-1), C.reshape(-1))
    res = run_bass_kernel_spmd(nc, in_maps, list(range(NCORES)))

    H = np.empty((FPAD, N), dtype=np.complex64)
    Hs = np.empty(FPAD, dtype=np.complex64)
    for c in range(NCORES):
        o = res.results[c]["H"].reshape(P, 16, T)
        s = res.results[c]["S"].reshape(P, 2, T)
        sl = slice(c * FC, (c + 1) * FC)
        for i in range(N):
            H[sl, i] = (o[:, 2 * i, :] + 1j * o[:, 2 * i + 1, :]).reshape(FC)
        Hs[sl] = (s[:, 0, :] + 1j * s[:, 1, :]).reshape(FC)
    H = H[:F]
    Hs = Hs[:F]

    h = np.fft.irfft(Hs.astype(np.complex128), n=NFFT)
    h = (h / np.max(np.abs(h))).astype(np.float32)
    return H.astype(np.complex64), h
